# revision 41
# baseline (speedup 1.0000x reference)
"""Multi-head attention (B=2, S=2048, D=2048, H=16) on 8 TRN2 NeuronCores.

Sharding: data-parallel over batch (2) x Megatron tensor-parallel over heads
(4 groups of 4 heads). Core c = 4*b + g handles batch b, heads [4g, 4g+4).
Host sums the 4 o_proj partials per batch and stacks the 2 batches.

Schedule (v3, all-bf16, single-pass x):
  Phase A: x streamed once in 4 granules of [P, 16dt, 512s]; per granule the
  k, v AND q projections all run (q no longer recomputed in phase B), outputs
  kept in SBUF as bf16: kT/qT [P, 4h, S], vv [P, 16st, DG]. All matmuls bf16
  (same PE rate as f32r, FWL weight loads, half the DMA/SBUF of the f32r
  baseline). Weights arrive on the scalar-engine DMA ring, x on the sync
  ring, so the first k-chain starts after ~400KB.

  Phase B per 512-query chunk, per head: scores matmuls write kt-PAIRS into a
  2-bank PSUM tile, ONE ACT exp covers the pair ([P,1024], amortizes the
  ~250ns ACT fixed cost), av matmul consumes each half (bf16). Softmax
  denominator: bf16 pair-sum tree on DVE (8+4+2 adds) then two ones-matmuls
  accumulate the partition reduction into a [128,512] PSUM tile -- every
  partition holds the sum, so reciprocal+normalize are full-width 128-lane
  DVE ops (the f32r baseline burned 54us of PE on per-kt ones-matmuls and
  3.3us/head on single-lane [1,512] reciprocals). o_proj bf16, out stored
  bf16 (host upcasts and sums partials in f32).

Per-core PE streaming floor is ~335us (qkv 164 + scores/av 109 + ones 1.7 +
o_proj 55 + DR none); ACT exp floor ~141us fully overlapped.

HW exec time (8-core SPMD): see test.py output.
"""

import math
import os

import ml_dtypes
import numpy as np

import concourse.mybir as mybir
import concourse.tile as tile
from concourse import bacc
from concourse.bass_utils import run_bass_kernel_spmd

F32 = mybir.dt.float32
BF16 = mybir.dt.bfloat16
EXP = mybir.ActivationFunctionType.Exp

B, S, D = 2, 2048, 2048
H = 16
HD = 128
G = 4
HLOC = H // G          # 4 heads per core
DG = HLOC * HD         # 512
P = 128
NCORES = 8

DT = D // P            # 16 contraction tiles
GR = 512               # x granule (s columns)
NG = S // GR           # 4
SCHUNK = 512
QC = S // SCHUNK       # 4 query chunks
KT = S // P            # 16 key tiles
MT = DG // HD          # 4 stationary column blocks per projection
IC = D // SCHUNK       # 4
ST = GR // P           # 4
INV_SQRT_HD = 1.0 / math.sqrt(HD)

_cache = {}
last_run = None  # BassKernelResults of the most recent execution (for test.py)


def build():
    nc = bacc.Bacc(None, target_bir_lowering=False)

    xP_dr = nc.dram_tensor("xP", [NG * P, DT * GR], BF16, kind="ExternalInput")
    wqT_dr = nc.dram_tensor("wqT", [MT * P, DT * HD], BF16, kind="ExternalInput")
    wkT_dr = nc.dram_tensor("wkT", [MT * P, DT * HD], BF16, kind="ExternalInput")
    wvT_dr = nc.dram_tensor("wvT", [D, DG], BF16, kind="ExternalInput")
    woT_dr = nc.dram_tensor("woT", [DG, D], BF16, kind="ExternalInput")
    out_d = nc.dram_tensor("out", [S, D], BF16, kind="ExternalOutput")

    xP_v = xP_dr.rearrange("(g p) (o s) -> g p o s", p=P, s=GR)
    wqT_v = wqT_dr.rearrange("(m p) (o h) -> m p o h", p=P, h=HD)
    wkT_v = wkT_dr.rearrange("(m p) (o h) -> m p o h", p=P, h=HD)
    wvT_v = wvT_dr.rearrange("(o p) m -> p o m", p=P)
    woT_v = woT_dr.rearrange("(o p) i -> p o i", p=P)

    with tile.TileContext(nc) as tc:
        with (
            tc.tile_pool(name="persist", bufs=1) as persist,
            tc.tile_pool(name="wA", bufs=1) as wA,
            tc.tile_pool(name="xs", bufs=2) as xpool,
            tc.tile_pool(name="expp", bufs=4) as expp,
            tc.tile_pool(name="accp", bufs=4) as accp,
            tc.tile_pool(name="ctxp", bufs=2) as ctxp,
            tc.tile_pool(name="small", bufs=2) as small,
            tc.tile_pool(name="ps", bufs=1, space="PSUM") as psp,
        ):
            kTt = persist.tile([P, HLOC, S], BF16, tag="kT")
            qTt = persist.tile([P, HLOC, S], BF16, tag="qT")
            vvt = persist.tile([P, KT, DG], BF16, tag="vv")
            wot = persist.tile([P, MT, D], BF16, tag="wo")
            onesb = persist.tile([P, P], BF16, tag="ones")
            ones32 = persist.tile([P, P], F32, tag="ones32")
            nc.vector.memset(ones32[:], 1.0)
            nc.vector.tensor_copy(onesb[:], ones32[:])

            # ~4us of tiny matmuls bridging the initial DMA wait: keeps the
            # PE-HAM activity window busy so the real chains start at 2.4GHz
            # instead of paying ~3.4us of half-clock warmup mid-phase-A.
            # Lives in the pso tag, which is idle until phase B -- it must
            # NOT share rotation with the phase-A ps512 accumulators.
            warm = psp.tile([P, SCHUNK], F32, tag="pso", bufs=2)
            for _ in range(28):
                nc.tensor.matmul(warm[0:64, 0:64], onesb[:, 0:64],
                                 onesb[:, 0:64], start=True, stop=True)

            wk_sb = wA.tile([P, MT, DT, HD], BF16, tag="wk")
            wq_sb = wA.tile([P, MT, DT, HD], BF16, tag="wq")
            wv_sb = wA.tile([P, DT, DG], BF16, tag="wv")

            # weights on the ACT DMA ring (idle until phase B), x on sync.
            # First k-chain's weights sliced fine so PE starts early.
            xg0 = xpool.tile([P, DT, GR], BF16, tag="xg")
            for d0 in range(0, DT, 2):
                nc.scalar.dma_start(wk_sb[:, 0, d0:d0 + 2], wkT_v[0, :, d0:d0 + 2])
            # granule 0's upper 10 dt-slices ride the scalar ring: the sync
            # ring's issue rate delivers ~6 slices by the time the ascending
            # k chain needs dt6, so dt6+ must come from the other ring
            for d0, w in [(6, 2), (8, 2), (10, 3), (13, 3)]:
                nc.scalar.dma_start(xg0[:, d0:d0 + w], xP_v[0, :, d0:d0 + w])
            for mt in range(1, MT):
                nc.scalar.dma_start(wk_sb[:, mt], wkT_v[mt])
            for d0 in range(0, DT, 4):
                nc.scalar.dma_start(wv_sb[:, d0:d0 + 4], wvT_v[:, d0:d0 + 4])
            for mt in range(MT):
                nc.scalar.dma_start(wq_sb[:, mt], wqT_v[mt])
            for jt in range(MT):
                nc.scalar.dma_start(wot[:, jt:jt + 1], woT_v[:, jt:jt + 1])

            # PE filler queue consumed between phase-B heads: weighted
            # closures (o_proj groups weight 1, deferred q-proj groups 4)
            pending = []

            def pop_filler(budget=4):
                while budget > 0 and pending:
                    w, fn = pending.pop(0)
                    fn()
                    budget -= w

            # ---------- phase A: k, v, q projections ----------
            for g in range(NG):
                if g == 0:
                    xg = xg0
                    for d0 in range(4):
                        nc.sync.dma_start(xg[:, d0:d0 + 1], xP_v[g, :, d0:d0 + 1])
                    nc.sync.dma_start(xg[:, 4:6], xP_v[g, :, 4:6])
                else:
                    xg = xpool.tile([P, DT, GR], BF16, tag="xg")
                    for d0 in range(0, DT, 4):
                        nc.sync.dma_start(xg[:, d0:d0 + 4], xP_v[g, :, d0:d0 + 4])

                for mt in range(MT):  # k projection
                    ps = psp.tile([P, GR], F32, tag="ps512", bufs=2)
                    for dt in range(DT):
                        nc.tensor.matmul(
                            ps[:], wk_sb[:, mt, dt, :], xg[:, dt, :],
                            start=(dt == 0), stop=(dt == DT - 1))
                        if g == 0 and mt == 0:
                            # no-dep filler: the first chain is DMA-paced,
                            # and a >3.4us PE idle here re-throttles HAM to
                            # 1.2GHz for the next 14us (observed) -- keep it
                            # busy (later chains run on resident x)
                            nc.tensor.matmul(warm[0:64, 0:64],
                                             onesb[:, 0:64], onesb[:, 0:64],
                                             start=True, stop=True)
                    nc.vector.tensor_copy(
                        kTt[:, mt, g * GR:(g + 1) * GR], ps[:])

                for st in range(ST):  # v projection
                    ps = psp.tile([P, DG], F32, tag="ps512", bufs=2)
                    for dt in range(DT):
                        nc.tensor.matmul(
                            ps[:], xg[:, dt, st * P:(st + 1) * P], wv_sb[:, dt, :],
                            start=(dt == 0), stop=(dt == DT - 1))
                    nc.vector.tensor_copy(vvt[:, g * ST + st, :], ps[:])

                def q_group(mt, xg=xg, g=g, tag="ps512"):
                    ps = psp.tile([P, GR], F32, tag=tag, bufs=2)
                    for dt in range(DT):
                        nc.tensor.matmul(
                            ps[:], wq_sb[:, mt, dt, :], xg[:, dt, :],
                            start=(dt == 0), stop=(dt == DT - 1))
                    nc.vector.tensor_copy(
                        qTt[:, mt, g * GR:(g + 1) * GR], ps[:])

                for mt in range(MT):  # q projection
                    q_group(mt)

            # ---------- phase B: attention + o_proj ----------
            def oproj_group(ctx_c, c, st, ic, alt=0):
                stile = c * ST + st
                if alt == 1:
                    # final drain: attention PSUM banks are idle, rotate
                    # through pss/pso too so consecutive groups pipeline
                    ps2 = psp.tile([P, 2, SCHUNK], F32, tag="pss", bufs=2)
                    ps = ps2[:, 0, :]
                elif alt == 2:
                    ps = psp.tile([P, SCHUNK], F32, tag="pso", bufs=2)
                else:
                    ps = psp.tile([P, SCHUNK], F32, tag="ps512", bufs=2)
                for jt in range(MT):
                    nc.tensor.matmul(
                        ps[:], ctx_c[:, jt, st * P:(st + 1) * P],
                        wot[:, jt, ic * SCHUNK:(ic + 1) * SCHUNK],
                        start=(jt == 0), stop=(jt == MT - 1))
                ob = small.tile([P, SCHUNK], BF16, tag="ostage", bufs=3)
                if alt:
                    # drain: DVE is the only busy engine left; ACT is idle
                    nc.scalar.copy(ob[:], ps[:])
                else:
                    nc.vector.tensor_copy(ob[:], ps[:])
                # sync HWDGE ring is idle in phase B (x loads done) and
                # avoids the multi-us SWDGE drain at teardown
                nc.sync.dma_start(
                    out_d[stile * P:(stile + 1) * P,
                          ic * SCHUNK:(ic + 1) * SCHUNK],
                    ob[:])

            # o_proj of chunk c-1 is issued interleaved between the heads of
            # chunk c: its ps512 tiles then rotate between the psden tiles
            # instead of queueing behind all four of them, and its matmuls
            # give PE filler work at every head boundary
            pending = []  # (ctx, c, st, ic) o_proj groups not yet issued
            wrapq = []    # (ctx, h, pso, l3) denominator wrap-ups not issued

            def wrapup():
                """ones-matmuls + reciprocal + normalize for the OLDEST
                pending head. Deferred by one head so the DVE tree is long
                done when the PE reaches the ones-matmuls (no PE stall)."""
                ctx_w, h_w, pso_w, l3_w = wrapq.pop(0)
                psden = psp.tile([P, SCHUNK], F32, tag="ps512", bufs=2)
                for i, t in enumerate(l3_w):
                    nc.tensor.matmul(psden[:], onesb[:], t[:],
                                     start=(i == 0), stop=(i == len(l3_w) - 1))
                rb = small.tile([P, SCHUNK], F32, tag="rb")
                # ~51 ULP is plenty for a softmax denominator; the exact
                # DVE reciprocal costs 3.4us/call and stalled the pso
                # PSUM pipeline
                nc.vector.reciprocal_approx_fast(rb[:], psden[:])
                nc.vector.tensor_mul(ctx_w[:, h_w, :], pso_w[:], rb[:])

            for c in range(QC):
                ctx = ctxp.tile([P, HLOC, SCHUNK], BF16, tag="ctx")
                for h in range(HLOC):
                    pso = psp.tile([P, SCHUNK], F32, tag="pso", bufs=2)
                    # av + denominator tree run one PAIR behind the scores
                    # so the PE never waits on ACT's ~1.1us pair-exp latency
                    l1, l2, l3 = [], [], []

                    def consume(kp_c, ex_c):
                        for j in range(2):
                            kt = 2 * kp_c + j
                            nc.tensor.matmul(
                                pso[:], vvt[:, kt, h * HD:(h + 1) * HD],
                                ex_c[:, j, :],
                                start=(kt == 0), stop=(kt == KT - 1))
                        t = accp.tile([P, SCHUNK], BF16, tag="pa1", bufs=4)
                        nc.vector.tensor_add(t[:], ex_c[:, 0, :], ex_c[:, 1, :])
                        l1.append(t)
                        if kp_c % 2 == 1:
                            t2 = accp.tile([P, SCHUNK], BF16, tag="pa2", bufs=4)
                            nc.vector.tensor_add(t2[:], l1[-2][:], l1[-1][:])
                            l2.append(t2)
                        if kp_c % 4 == 3:
                            t3 = accp.tile([P, SCHUNK], BF16, tag="pa3", bufs=4)
                            nc.vector.tensor_add(t3[:], l2[-2][:], l2[-1][:])
                            l3.append(t3)

                    prev = None
                    for kp in range(KT // 2):
                        pss = psp.tile([P, 2, SCHUNK], F32, tag="pss", bufs=2)
                        for j in range(2):
                            kt = 2 * kp + j
                            nc.tensor.matmul(
                                pss[:, j, :],
                                kTt[:, h, kt * P:(kt + 1) * P],
                                qTt[:, h, c * SCHUNK:(c + 1) * SCHUNK],
                                start=True, stop=True)
                        ex = expp.tile([P, 2, SCHUNK], BF16, tag="expP",
                                       bufs=6)
                        nc.scalar.activation(ex[:], pss[:], EXP,
                                             scale=INV_SQRT_HD)
                        if prev is not None:
                            consume(*prev)
                        prev = (kp, ex)

                    # wrap-up of the PREVIOUS head (its tree is a full head
                    # old -> no PE stall) + o_proj filler of the previous
                    # chunk bridge the last pair's ACT latency too
                    if wrapq:
                        wrapup()
                    pop_filler(4)
                    consume(*prev)
                    wrapq.append((ctx, h, pso, l3))

                for st in range(ST):
                    for ic in range(IC):
                        pending.append((1, lambda alt=False, ctx=ctx, c=c,
                                        st=st, ic=ic:
                                        oproj_group(ctx, c, st, ic, alt)))
            wrapup()  # chunk 3 head 3
            drain = 0
            while pending:
                _, fn = pending.pop(0)
                fn(alt=(drain % 3))
                drain += 1

    nc.finalize()
    return nc


def _bf16(a):
    return np.asarray(a, dtype=np.float32).astype(ml_dtypes.bfloat16)


def _pack_x(xT):
    """[D, S] bf16 -> [NG*P, DT*GR] blocked so device DMAs are contiguous."""
    return np.ascontiguousarray(
        xT.reshape(DT, P, NG, GR).transpose(2, 1, 0, 3)
    ).reshape(NG * P, DT * GR)


def _pack_w_mt(wT):
    """[D, DG] bf16 -> [MT*P, DT*HD] blocked by stationary column block."""
    return np.ascontiguousarray(
        wT.reshape(DT, P, MT, HD).transpose(2, 1, 0, 3)
    ).reshape(MT * P, DT * HD)


def kernel(hidden_states, wq, wk, wv, wo):
    global last_run
    if "nc" not in _cache:
        _cache["nc"] = build()
    nc = _cache["nc"]

    hidden_states = np.asarray(hidden_states, dtype=np.float32)
    wq = np.asarray(wq, dtype=np.float32)
    wk = np.asarray(wk, dtype=np.float32)
    wv = np.asarray(wv, dtype=np.float32)
    wo = np.asarray(wo, dtype=np.float32)

    xP = [_pack_x(_bf16(hidden_states[b].T)) for b in range(B)]
    in_maps = []
    for c in range(NCORES):
        b, g = divmod(c, G)
        sl = slice(g * DG, (g + 1) * DG)
        in_maps.append({
            "xP": xP[b],
            "wqT": _pack_w_mt(_bf16(wq[sl, :].T)),
            "wkT": _pack_w_mt(_bf16(wk[sl, :].T)),
            "wvT": _bf16(wv[sl, :].T),
            "woT": _bf16(wo[:, sl].T),
        })

    trace = os.environ.get("BASSKERNEL_TRACE", "0") == "1"
    last_run = run_bass_kernel_spmd(
        nc, in_maps, core_ids=list(range(NCORES)), trace=trace)

    out = np.empty((B, S, D), dtype=np.float32)
    for b in range(B):
        acc = None
        for g in range(G):
            part = last_run.results[b * G + g]["out"].astype(np.float32)
            acc = part if acc is None else acc + part
        out[b] = acc
    return out


# revision 42
# speedup vs baseline: 1.0061x; 1.0061x over previous
"""Multi-head attention (B=2, S=2048, D=2048, H=16) on 8 TRN2 NeuronCores.

Sharding: data-parallel over batch (2) x Megatron tensor-parallel over heads
(4 groups of 4 heads). Core c = 4*b + g handles batch b, heads [4g, 4g+4).
Host sums the 4 o_proj partials per batch and stacks the 2 batches.

Schedule (v3, all-bf16, single-pass x):
  Phase A: x streamed once in 4 granules of [P, 16dt, 512s]; per granule the
  k, v AND q projections all run (q no longer recomputed in phase B), outputs
  kept in SBUF as bf16: kT/qT [P, 4h, S], vv [P, 16st, DG]. All matmuls bf16
  (same PE rate as f32r, FWL weight loads, half the DMA/SBUF of the f32r
  baseline). Weights arrive on the scalar-engine DMA ring, x on the sync
  ring, so the first k-chain starts after ~400KB.

  Phase B per 512-query chunk, per head: scores matmuls write kt-PAIRS into a
  2-bank PSUM tile, ONE ACT exp covers the pair ([P,1024], amortizes the
  ~250ns ACT fixed cost), av matmul consumes each half (bf16). Softmax
  denominator: bf16 pair-sum tree on DVE (8+4+2 adds) then two ones-matmuls
  accumulate the partition reduction into a [128,512] PSUM tile -- every
  partition holds the sum, so reciprocal+normalize are full-width 128-lane
  DVE ops (the f32r baseline burned 54us of PE on per-kt ones-matmuls and
  3.3us/head on single-lane [1,512] reciprocals). o_proj bf16, out stored
  bf16 (host upcasts and sums partials in f32).

Per-core PE streaming floor is ~335us (qkv 164 + scores/av 109 + ones 1.7 +
o_proj 55 + DR none); ACT exp floor ~141us fully overlapped.

HW exec time (8-core SPMD): see test.py output.
"""

import math
import os

import ml_dtypes
import numpy as np

import concourse.mybir as mybir
import concourse.tile as tile
from concourse import bacc
from concourse.bass_utils import run_bass_kernel_spmd

F32 = mybir.dt.float32
BF16 = mybir.dt.bfloat16
EXP = mybir.ActivationFunctionType.Exp

B, S, D = 2, 2048, 2048
H = 16
HD = 128
G = 4
HLOC = H // G          # 4 heads per core
DG = HLOC * HD         # 512
P = 128
NCORES = 8

DT = D // P            # 16 contraction tiles
GR = 512               # x granule (s columns)
NG = S // GR           # 4
SCHUNK = 512
QC = S // SCHUNK       # 4 query chunks
KT = S // P            # 16 key tiles
MT = DG // HD          # 4 stationary column blocks per projection
IC = D // SCHUNK       # 4
ST = GR // P           # 4
INV_SQRT_HD = 1.0 / math.sqrt(HD)

_cache = {}
last_run = None  # BassKernelResults of the most recent execution (for test.py)


def build():
    nc = bacc.Bacc(None, target_bir_lowering=False)

    xP_dr = nc.dram_tensor("xP", [NG * P, DT * GR], BF16, kind="ExternalInput")
    wqT_dr = nc.dram_tensor("wqT", [MT * P, DT * HD], BF16, kind="ExternalInput")
    wkT_dr = nc.dram_tensor("wkT", [MT * P, DT * HD], BF16, kind="ExternalInput")
    wvT_dr = nc.dram_tensor("wvT", [D, DG], BF16, kind="ExternalInput")
    woT_dr = nc.dram_tensor("woT", [DG, D], BF16, kind="ExternalInput")
    out_d = nc.dram_tensor("out", [S, D], BF16, kind="ExternalOutput")

    xP_v = xP_dr.rearrange("(g p) (o s) -> g p o s", p=P, s=GR)
    wqT_v = wqT_dr.rearrange("(m p) (o h) -> m p o h", p=P, h=HD)
    wkT_v = wkT_dr.rearrange("(m p) (o h) -> m p o h", p=P, h=HD)
    wvT_v = wvT_dr.rearrange("(o p) m -> p o m", p=P)
    woT_v = woT_dr.rearrange("(o p) i -> p o i", p=P)

    with tile.TileContext(nc) as tc:
        with (
            tc.tile_pool(name="persist", bufs=1) as persist,
            tc.tile_pool(name="wA", bufs=1) as wA,
            tc.tile_pool(name="xs", bufs=2) as xpool,
            tc.tile_pool(name="expp", bufs=4) as expp,
            tc.tile_pool(name="accp", bufs=4) as accp,
            tc.tile_pool(name="ctxp", bufs=2) as ctxp,
            tc.tile_pool(name="small", bufs=2) as small,
            tc.tile_pool(name="ps", bufs=1, space="PSUM") as psp,
        ):
            kTt = persist.tile([P, HLOC, S], BF16, tag="kT")
            qTt = persist.tile([P, HLOC, S], BF16, tag="qT")
            vvt = persist.tile([P, KT, DG], BF16, tag="vv")
            wot = persist.tile([P, MT, D], BF16, tag="wo")
            onesb = persist.tile([P, P], BF16, tag="ones")
            ones32 = persist.tile([P, P], F32, tag="ones32")
            nc.vector.memset(ones32[:], 1.0)
            nc.vector.tensor_copy(onesb[:], ones32[:])

            # ~4us of tiny matmuls bridging the initial DMA wait: keeps the
            # PE-HAM activity window busy so the real chains start at 2.4GHz
            # instead of paying ~3.4us of half-clock warmup mid-phase-A.
            # Lives in the pso tag, which is idle until phase B -- it must
            # NOT share rotation with the phase-A ps512 accumulators.
            warm = psp.tile([P, SCHUNK], F32, tag="pso", bufs=2)
            for _ in range(28):
                nc.tensor.matmul(warm[0:64, 0:64], onesb[:, 0:64],
                                 onesb[:, 0:64], start=True, stop=True)

            wk_sb = wA.tile([P, MT, DT, HD], BF16, tag="wk")
            wq_sb = wA.tile([P, MT, DT, HD], BF16, tag="wq")
            wv_sb = wA.tile([P, DT, DG], BF16, tag="wv")

            # weights on the ACT DMA ring (idle until phase B), x on sync.
            # First k-chain's weights sliced fine so PE starts early.
            xg0 = xpool.tile([P, DT, GR], BF16, tag="xg")
            # wk mt0 in 2 coarse slices: the chain-major k loop only needs
            # dt0 early, and 8 fine slices cost 6.4us of issue time that
            # delays the x dt6+ slices queued behind them on this ring
            nc.scalar.dma_start(wk_sb[:, 0, 0:8], wkT_v[0, :, 0:8])
            nc.scalar.dma_start(wk_sb[:, 0, 8:16], wkT_v[0, :, 8:16])
            # granule 0's upper 10 dt-slices ride the scalar ring: the sync
            # ring's issue rate delivers ~6 slices by the time the ascending
            # k chain needs dt6, so dt6+ must come from the other ring
            for d0, w in [(6, 2), (8, 2), (10, 3), (13, 3)]:
                nc.scalar.dma_start(xg0[:, d0:d0 + w], xP_v[0, :, d0:d0 + w])
            for mt in range(1, MT):
                nc.scalar.dma_start(wk_sb[:, mt], wkT_v[mt])
            for d0 in range(0, DT, 4):
                nc.scalar.dma_start(wv_sb[:, d0:d0 + 4], wvT_v[:, d0:d0 + 4])
            for mt in range(MT):
                nc.scalar.dma_start(wq_sb[:, mt], wqT_v[mt])
            for jt in range(MT):
                nc.scalar.dma_start(wot[:, jt:jt + 1], woT_v[:, jt:jt + 1])

            # PE filler queue consumed between phase-B heads: weighted
            # closures (o_proj groups weight 1, deferred q-proj groups 4)
            pending = []

            def pop_filler(budget=4):
                while budget > 0 and pending:
                    w, fn = pending.pop(0)
                    fn()
                    budget -= w

            # ---------- phase A: k, v, q projections ----------
            for g in range(NG):
                if g == 0:
                    xg = xg0
                    for d0 in range(4):
                        nc.sync.dma_start(xg[:, d0:d0 + 1], xP_v[g, :, d0:d0 + 1])
                    nc.sync.dma_start(xg[:, 4:6], xP_v[g, :, 4:6])
                else:
                    xg = xpool.tile([P, DT, GR], BF16, tag="xg")
                    for d0 in range(0, DT, 4):
                        nc.sync.dma_start(xg[:, d0:d0 + 4], xP_v[g, :, d0:d0 + 4])

                for mt in range(MT):  # k projection
                    ps = psp.tile([P, GR], F32, tag="ps512", bufs=2)
                    for dt in range(DT):
                        nc.tensor.matmul(
                            ps[:], wk_sb[:, mt, dt, :], xg[:, dt, :],
                            start=(dt == 0), stop=(dt == DT - 1))
                        if g == 0 and mt == 0:
                            # no-dep filler: the first chain is DMA-paced,
                            # and a >3.4us PE idle here re-throttles HAM to
                            # 1.2GHz for the next 14us (observed) -- keep it
                            # busy (later chains run on resident x)
                            nc.tensor.matmul(warm[0:64, 0:64],
                                             onesb[:, 0:64], onesb[:, 0:64],
                                             start=True, stop=True)
                    nc.vector.tensor_copy(
                        kTt[:, mt, g * GR:(g + 1) * GR], ps[:])

                for st in range(ST):  # v projection
                    ps = psp.tile([P, DG], F32, tag="ps512", bufs=2)
                    for dt in range(DT):
                        nc.tensor.matmul(
                            ps[:], xg[:, dt, st * P:(st + 1) * P], wv_sb[:, dt, :],
                            start=(dt == 0), stop=(dt == DT - 1))
                    nc.vector.tensor_copy(vvt[:, g * ST + st, :], ps[:])

                def q_group(mt, xg=xg, g=g, tag="ps512"):
                    ps = psp.tile([P, GR], F32, tag=tag, bufs=2)
                    for dt in range(DT):
                        nc.tensor.matmul(
                            ps[:], wq_sb[:, mt, dt, :], xg[:, dt, :],
                            start=(dt == 0), stop=(dt == DT - 1))
                    nc.vector.tensor_copy(
                        qTt[:, mt, g * GR:(g + 1) * GR], ps[:])

                for mt in range(MT):  # q projection
                    q_group(mt)

            # ---------- phase B: attention + o_proj ----------
            def oproj_group(ctx_c, c, st, ic, alt=0):
                stile = c * ST + st
                if alt == 1:
                    # final drain: attention PSUM banks are idle, rotate
                    # through pss/pso too so consecutive groups pipeline
                    ps2 = psp.tile([P, 2, SCHUNK], F32, tag="pss", bufs=2)
                    ps = ps2[:, 0, :]
                elif alt == 2:
                    ps = psp.tile([P, SCHUNK], F32, tag="pso", bufs=2)
                else:
                    ps = psp.tile([P, SCHUNK], F32, tag="ps512", bufs=2)
                for jt in range(MT):
                    nc.tensor.matmul(
                        ps[:], ctx_c[:, jt, st * P:(st + 1) * P],
                        wot[:, jt, ic * SCHUNK:(ic + 1) * SCHUNK],
                        start=(jt == 0), stop=(jt == MT - 1))
                ob = small.tile([P, SCHUNK], BF16, tag="ostage", bufs=3)
                if alt:
                    # drain: DVE is the only busy engine left; ACT is idle
                    nc.scalar.copy(ob[:], ps[:])
                else:
                    nc.vector.tensor_copy(ob[:], ps[:])
                # sync HWDGE ring is idle in phase B (x loads done) and
                # avoids the multi-us SWDGE drain at teardown
                nc.sync.dma_start(
                    out_d[stile * P:(stile + 1) * P,
                          ic * SCHUNK:(ic + 1) * SCHUNK],
                    ob[:])

            # o_proj of chunk c-1 is issued interleaved between the heads of
            # chunk c: its ps512 tiles then rotate between the psden tiles
            # instead of queueing behind all four of them, and its matmuls
            # give PE filler work at every head boundary
            pending = []  # (ctx, c, st, ic) o_proj groups not yet issued
            wrapq = []    # (ctx, h, pso, l3) denominator wrap-ups not issued

            def wrapup():
                """ones-matmuls + reciprocal + normalize for the OLDEST
                pending head. Deferred by one head so the DVE tree is long
                done when the PE reaches the ones-matmuls (no PE stall)."""
                ctx_w, h_w, pso_w, l3_w = wrapq.pop(0)
                psden = psp.tile([P, SCHUNK], F32, tag="ps512", bufs=2)
                for i, t in enumerate(l3_w):
                    nc.tensor.matmul(psden[:], onesb[:], t[:],
                                     start=(i == 0), stop=(i == len(l3_w) - 1))
                rb = small.tile([P, SCHUNK], F32, tag="rb")
                # ~51 ULP is plenty for a softmax denominator; the exact
                # DVE reciprocal costs 3.4us/call and stalled the pso
                # PSUM pipeline
                nc.vector.reciprocal_approx_fast(rb[:], psden[:])
                nc.vector.tensor_mul(ctx_w[:, h_w, :], pso_w[:], rb[:])

            for c in range(QC):
                ctx = ctxp.tile([P, HLOC, SCHUNK], BF16, tag="ctx")
                for h in range(HLOC):
                    pso = psp.tile([P, SCHUNK], F32, tag="pso", bufs=2)
                    # av + denominator tree run one PAIR behind the scores
                    # so the PE never waits on ACT's ~1.1us pair-exp latency
                    l1, l2, l3 = [], [], []

                    def consume(kp_c, ex_c):
                        for j in range(2):
                            kt = 2 * kp_c + j
                            nc.tensor.matmul(
                                pso[:], vvt[:, kt, h * HD:(h + 1) * HD],
                                ex_c[:, j, :],
                                start=(kt == 0), stop=(kt == KT - 1))
                        t = accp.tile([P, SCHUNK], BF16, tag="pa1", bufs=4)
                        nc.vector.tensor_add(t[:], ex_c[:, 0, :], ex_c[:, 1, :])
                        l1.append(t)
                        if kp_c % 2 == 1:
                            t2 = accp.tile([P, SCHUNK], BF16, tag="pa2", bufs=4)
                            nc.vector.tensor_add(t2[:], l1[-2][:], l1[-1][:])
                            l2.append(t2)
                        if kp_c % 4 == 3:
                            t3 = accp.tile([P, SCHUNK], BF16, tag="pa3", bufs=4)
                            nc.vector.tensor_add(t3[:], l2[-2][:], l2[-1][:])
                            l3.append(t3)

                    prev = None
                    for kp in range(KT // 2):
                        pss = psp.tile([P, 2, SCHUNK], F32, tag="pss", bufs=2)
                        for j in range(2):
                            kt = 2 * kp + j
                            nc.tensor.matmul(
                                pss[:, j, :],
                                kTt[:, h, kt * P:(kt + 1) * P],
                                qTt[:, h, c * SCHUNK:(c + 1) * SCHUNK],
                                start=True, stop=True)
                        ex = expp.tile([P, 2, SCHUNK], BF16, tag="expP",
                                       bufs=6)
                        nc.scalar.activation(ex[:], pss[:], EXP,
                                             scale=INV_SQRT_HD)
                        if prev is not None:
                            consume(*prev)
                        prev = (kp, ex)

                    # wrap-up of the PREVIOUS head (its tree is a full head
                    # old -> no PE stall) + o_proj filler of the previous
                    # chunk bridge the last pair's ACT latency too
                    if wrapq:
                        wrapup()
                    pop_filler(4)
                    consume(*prev)
                    wrapq.append((ctx, h, pso, l3))

                for st in range(ST):
                    for ic in range(IC):
                        pending.append((1, lambda alt=False, ctx=ctx, c=c,
                                        st=st, ic=ic:
                                        oproj_group(ctx, c, st, ic, alt)))
            wrapup()  # chunk 3 head 3
            drain = 0
            while pending:
                _, fn = pending.pop(0)
                fn(alt=(drain % 3))
                drain += 1

    nc.finalize()
    return nc


def _bf16(a):
    return np.asarray(a, dtype=np.float32).astype(ml_dtypes.bfloat16)


def _pack_x(xT):
    """[D, S] bf16 -> [NG*P, DT*GR] blocked so device DMAs are contiguous."""
    return np.ascontiguousarray(
        xT.reshape(DT, P, NG, GR).transpose(2, 1, 0, 3)
    ).reshape(NG * P, DT * GR)


def _pack_w_mt(wT):
    """[D, DG] bf16 -> [MT*P, DT*HD] blocked by stationary column block."""
    return np.ascontiguousarray(
        wT.reshape(DT, P, MT, HD).transpose(2, 1, 0, 3)
    ).reshape(MT * P, DT * HD)


def kernel(hidden_states, wq, wk, wv, wo):
    global last_run
    if "nc" not in _cache:
        _cache["nc"] = build()
    nc = _cache["nc"]

    hidden_states = np.asarray(hidden_states, dtype=np.float32)
    wq = np.asarray(wq, dtype=np.float32)
    wk = np.asarray(wk, dtype=np.float32)
    wv = np.asarray(wv, dtype=np.float32)
    wo = np.asarray(wo, dtype=np.float32)

    xP = [_pack_x(_bf16(hidden_states[b].T)) for b in range(B)]
    in_maps = []
    for c in range(NCORES):
        b, g = divmod(c, G)
        sl = slice(g * DG, (g + 1) * DG)
        in_maps.append({
            "xP": xP[b],
            "wqT": _pack_w_mt(_bf16(wq[sl, :].T)),
            "wkT": _pack_w_mt(_bf16(wk[sl, :].T)),
            "wvT": _bf16(wv[sl, :].T),
            "woT": _bf16(wo[:, sl].T),
        })

    trace = os.environ.get("BASSKERNEL_TRACE", "0") == "1"
    last_run = run_bass_kernel_spmd(
        nc, in_maps, core_ids=list(range(NCORES)), trace=trace)

    out = np.empty((B, S, D), dtype=np.float32)
    for b in range(B):
        acc = None
        for g in range(G):
            part = last_run.results[b * G + g]["out"].astype(np.float32)
            acc = part if acc is None else acc + part
        out[b] = acc
    return out


# revision 43
# speedup vs baseline: 1.0091x; 1.0029x over previous
"""Multi-head attention (B=2, S=2048, D=2048, H=16) on 8 TRN2 NeuronCores.

Sharding: data-parallel over batch (2) x Megatron tensor-parallel over heads
(4 groups of 4 heads). Core c = 4*b + g handles batch b, heads [4g, 4g+4).
Host sums the 4 o_proj partials per batch and stacks the 2 batches.

Schedule (v3, all-bf16, single-pass x):
  Phase A: x streamed once in 4 granules of [P, 16dt, 512s]; per granule the
  k, v AND q projections all run (q no longer recomputed in phase B), outputs
  kept in SBUF as bf16: kT/qT [P, 4h, S], vv [P, 16st, DG]. All matmuls bf16
  (same PE rate as f32r, FWL weight loads, half the DMA/SBUF of the f32r
  baseline). Weights arrive on the scalar-engine DMA ring, x on the sync
  ring, so the first k-chain starts after ~400KB.

  Phase B per 512-query chunk, per head: scores matmuls write kt-PAIRS into a
  2-bank PSUM tile, ONE ACT exp covers the pair ([P,1024], amortizes the
  ~250ns ACT fixed cost), av matmul consumes each half (bf16). Softmax
  denominator: bf16 pair-sum tree on DVE (8+4+2 adds) then two ones-matmuls
  accumulate the partition reduction into a [128,512] PSUM tile -- every
  partition holds the sum, so reciprocal+normalize are full-width 128-lane
  DVE ops (the f32r baseline burned 54us of PE on per-kt ones-matmuls and
  3.3us/head on single-lane [1,512] reciprocals). o_proj bf16, out stored
  bf16 (host upcasts and sums partials in f32).

Per-core PE streaming floor is ~335us (qkv 164 + scores/av 109 + ones 1.7 +
o_proj 55 + DR none); ACT exp floor ~141us fully overlapped.

HW exec time (8-core SPMD): see test.py output.
"""

import math
import os

import ml_dtypes
import numpy as np

import concourse.mybir as mybir
import concourse.tile as tile
from concourse import bacc
from concourse.bass_utils import run_bass_kernel_spmd

F32 = mybir.dt.float32
BF16 = mybir.dt.bfloat16
EXP = mybir.ActivationFunctionType.Exp

B, S, D = 2, 2048, 2048
H = 16
HD = 128
G = 4
HLOC = H // G          # 4 heads per core
DG = HLOC * HD         # 512
P = 128
NCORES = 8

DT = D // P            # 16 contraction tiles
GR = 512               # x granule (s columns)
NG = S // GR           # 4
SCHUNK = 512
QC = S // SCHUNK       # 4 query chunks
KT = S // P            # 16 key tiles
MT = DG // HD          # 4 stationary column blocks per projection
IC = D // SCHUNK       # 4
ST = GR // P           # 4
INV_SQRT_HD = 1.0 / math.sqrt(HD)

_cache = {}
last_run = None  # BassKernelResults of the most recent execution (for test.py)


def build():
    nc = bacc.Bacc(None, target_bir_lowering=False)

    xP_dr = nc.dram_tensor("xP", [NG * P, DT * GR], BF16, kind="ExternalInput")
    wqT_dr = nc.dram_tensor("wqT", [MT * P, DT * HD], BF16, kind="ExternalInput")
    wkT_dr = nc.dram_tensor("wkT", [MT * P, DT * HD], BF16, kind="ExternalInput")
    wvT_dr = nc.dram_tensor("wvT", [D, DG], BF16, kind="ExternalInput")
    woT_dr = nc.dram_tensor("woT", [DG, D], BF16, kind="ExternalInput")
    out_d = nc.dram_tensor("out", [S, D], BF16, kind="ExternalOutput")

    xP_v = xP_dr.rearrange("(g p) (o s) -> g p o s", p=P, s=GR)
    wqT_v = wqT_dr.rearrange("(m p) (o h) -> m p o h", p=P, h=HD)
    wkT_v = wkT_dr.rearrange("(m p) (o h) -> m p o h", p=P, h=HD)
    wvT_v = wvT_dr.rearrange("(o p) m -> p o m", p=P)
    woT_v = woT_dr.rearrange("(o p) i -> p o i", p=P)

    with tile.TileContext(nc) as tc:
        with (
            tc.tile_pool(name="persist", bufs=1) as persist,
            tc.tile_pool(name="wA", bufs=1) as wA,
            tc.tile_pool(name="xs", bufs=2) as xpool,
            tc.tile_pool(name="expp", bufs=4) as expp,
            tc.tile_pool(name="accp", bufs=4) as accp,
            tc.tile_pool(name="ctxp", bufs=2) as ctxp,
            tc.tile_pool(name="small", bufs=2) as small,
            tc.tile_pool(name="ps", bufs=1, space="PSUM") as psp,
        ):
            kTt = persist.tile([P, HLOC, S], BF16, tag="kT")
            qTt = persist.tile([P, HLOC, S], BF16, tag="qT")
            vvt = persist.tile([P, KT, DG], BF16, tag="vv")
            wot = persist.tile([P, MT, D], BF16, tag="wo")
            onesb = persist.tile([P, P], BF16, tag="ones")
            ones32 = persist.tile([P, P], F32, tag="ones32")
            nc.vector.memset(ones32[:], 1.0)
            nc.vector.tensor_copy(onesb[:], ones32[:])

            # ~4us of tiny matmuls bridging the initial DMA wait: keeps the
            # PE-HAM activity window busy so the real chains start at 2.4GHz
            # instead of paying ~3.4us of half-clock warmup mid-phase-A.
            # Lives in the pso tag, which is idle until phase B -- it must
            # NOT share rotation with the phase-A ps512 accumulators.
            # 40 x ~107ns (cold) = ~4.3us of continuous PE activity: enough
            # to cross the HAM 3.41us SHORT window so the un-throttle to
            # 2.4GHz fires before the first real chains start (28 was
            # measured to stay just under the window -> cold until ~30us)
            warm = psp.tile([P, SCHUNK], F32, tag="pso", bufs=2)
            for _ in range(40):
                nc.tensor.matmul(warm[0:64, 0:64], onesb[:, 0:64],
                                 onesb[:, 0:64], start=True, stop=True)

            wk_sb = wA.tile([P, MT, DT, HD], BF16, tag="wk")
            wq_sb = wA.tile([P, MT, DT, HD], BF16, tag="wq")
            wv_sb = wA.tile([P, DT, DG], BF16, tag="wv")

            # weights on the ACT DMA ring (idle until phase B), x on sync.
            # First k-chain's weights sliced fine so PE starts early.
            xg0 = xpool.tile([P, DT, GR], BF16, tag="xg")
            # wk mt0 in 2 coarse slices: the chain-major k loop only needs
            # dt0 early, and 8 fine slices cost 6.4us of issue time that
            # delays the x dt6+ slices queued behind them on this ring
            nc.scalar.dma_start(wk_sb[:, 0, 0:8], wkT_v[0, :, 0:8])
            nc.scalar.dma_start(wk_sb[:, 0, 8:16], wkT_v[0, :, 8:16])
            # granule 0's upper 10 dt-slices ride the scalar ring: the sync
            # ring's issue rate delivers ~6 slices by the time the ascending
            # k chain needs dt6, so dt6+ must come from the other ring
            for d0, w in [(6, 2), (8, 2), (10, 3), (13, 3)]:
                nc.scalar.dma_start(xg0[:, d0:d0 + w], xP_v[0, :, d0:d0 + w])
            for mt in range(1, MT):
                nc.scalar.dma_start(wk_sb[:, mt], wkT_v[mt])
            for d0 in range(0, DT, 4):
                nc.scalar.dma_start(wv_sb[:, d0:d0 + 4], wvT_v[:, d0:d0 + 4])
            for mt in range(MT):
                nc.scalar.dma_start(wq_sb[:, mt], wqT_v[mt])
            for jt in range(MT):
                nc.scalar.dma_start(wot[:, jt:jt + 1], woT_v[:, jt:jt + 1])

            # PE filler queue consumed between phase-B heads: weighted
            # closures (o_proj groups weight 1, deferred q-proj groups 4)
            pending = []

            def pop_filler(budget=4):
                while budget > 0 and pending:
                    w, fn = pending.pop(0)
                    fn()
                    budget -= w

            # ---------- phase A: k, v, q projections ----------
            for g in range(NG):
                if g == 0:
                    xg = xg0
                    for d0 in range(4):
                        nc.sync.dma_start(xg[:, d0:d0 + 1], xP_v[g, :, d0:d0 + 1])
                    nc.sync.dma_start(xg[:, 4:6], xP_v[g, :, 4:6])
                else:
                    xg = xpool.tile([P, DT, GR], BF16, tag="xg")
                    for d0 in range(0, DT, 4):
                        nc.sync.dma_start(xg[:, d0:d0 + 4], xP_v[g, :, d0:d0 + 4])

                for mt in range(MT):  # k projection
                    ps = psp.tile([P, GR], F32, tag="ps512", bufs=2)
                    for dt in range(DT):
                        nc.tensor.matmul(
                            ps[:], wk_sb[:, mt, dt, :], xg[:, dt, :],
                            start=(dt == 0), stop=(dt == DT - 1))
                        if g == 0 and mt == 0:
                            # no-dep filler: the first chain is DMA-paced,
                            # and a >3.4us PE idle here re-throttles HAM to
                            # 1.2GHz for the next 14us (observed) -- keep it
                            # busy (later chains run on resident x)
                            nc.tensor.matmul(warm[0:64, 0:64],
                                             onesb[:, 0:64], onesb[:, 0:64],
                                             start=True, stop=True)
                    nc.vector.tensor_copy(
                        kTt[:, mt, g * GR:(g + 1) * GR], ps[:])

                for st in range(ST):  # v projection
                    ps = psp.tile([P, DG], F32, tag="ps512", bufs=2)
                    for dt in range(DT):
                        nc.tensor.matmul(
                            ps[:], xg[:, dt, st * P:(st + 1) * P], wv_sb[:, dt, :],
                            start=(dt == 0), stop=(dt == DT - 1))
                    nc.vector.tensor_copy(vvt[:, g * ST + st, :], ps[:])

                def q_group(mt, xg=xg, g=g, tag="ps512"):
                    ps = psp.tile([P, GR], F32, tag=tag, bufs=2)
                    for dt in range(DT):
                        nc.tensor.matmul(
                            ps[:], wq_sb[:, mt, dt, :], xg[:, dt, :],
                            start=(dt == 0), stop=(dt == DT - 1))
                    nc.vector.tensor_copy(
                        qTt[:, mt, g * GR:(g + 1) * GR], ps[:])

                for mt in range(MT):  # q projection
                    q_group(mt)

            # ---------- phase B: attention + o_proj ----------
            def oproj_group(ctx_c, c, st, ic, alt=0):
                stile = c * ST + st
                if alt == 1:
                    # final drain: attention PSUM banks are idle, rotate
                    # through pss/pso too so consecutive groups pipeline
                    ps2 = psp.tile([P, 2, SCHUNK], F32, tag="pss", bufs=2)
                    ps = ps2[:, 0, :]
                elif alt == 2:
                    ps = psp.tile([P, SCHUNK], F32, tag="pso", bufs=2)
                else:
                    ps = psp.tile([P, SCHUNK], F32, tag="ps512", bufs=2)
                for jt in range(MT):
                    nc.tensor.matmul(
                        ps[:], ctx_c[:, jt, st * P:(st + 1) * P],
                        wot[:, jt, ic * SCHUNK:(ic + 1) * SCHUNK],
                        start=(jt == 0), stop=(jt == MT - 1))
                ob = small.tile([P, SCHUNK], BF16, tag="ostage", bufs=3)
                if alt:
                    # drain: DVE is the only busy engine left; ACT is idle
                    nc.scalar.copy(ob[:], ps[:])
                else:
                    nc.vector.tensor_copy(ob[:], ps[:])
                # sync HWDGE ring is idle in phase B (x loads done) and
                # avoids the multi-us SWDGE drain at teardown
                nc.sync.dma_start(
                    out_d[stile * P:(stile + 1) * P,
                          ic * SCHUNK:(ic + 1) * SCHUNK],
                    ob[:])

            # o_proj of chunk c-1 is issued interleaved between the heads of
            # chunk c: its ps512 tiles then rotate between the psden tiles
            # instead of queueing behind all four of them, and its matmuls
            # give PE filler work at every head boundary
            pending = []  # (ctx, c, st, ic) o_proj groups not yet issued
            wrapq = []    # (ctx, h, pso, l3) denominator wrap-ups not issued

            def wrapup():
                """ones-matmuls + reciprocal + normalize for the OLDEST
                pending head. Deferred by one head so the DVE tree is long
                done when the PE reaches the ones-matmuls (no PE stall)."""
                ctx_w, h_w, pso_w, l3_w = wrapq.pop(0)
                psden = psp.tile([P, SCHUNK], F32, tag="ps512", bufs=2)
                for i, t in enumerate(l3_w):
                    nc.tensor.matmul(psden[:], onesb[:], t[:],
                                     start=(i == 0), stop=(i == len(l3_w) - 1))
                rb = small.tile([P, SCHUNK], F32, tag="rb")
                # ~51 ULP is plenty for a softmax denominator; the exact
                # DVE reciprocal costs 3.4us/call and stalled the pso
                # PSUM pipeline
                nc.vector.reciprocal_approx_fast(rb[:], psden[:])
                nc.vector.tensor_mul(ctx_w[:, h_w, :], pso_w[:], rb[:])

            for c in range(QC):
                ctx = ctxp.tile([P, HLOC, SCHUNK], BF16, tag="ctx")
                for h in range(HLOC):
                    pso = psp.tile([P, SCHUNK], F32, tag="pso", bufs=2)
                    # av + denominator tree run one PAIR behind the scores
                    # so the PE never waits on ACT's ~1.1us pair-exp latency
                    l1, l2, l3 = [], [], []

                    def consume(kp_c, ex_c):
                        for j in range(2):
                            kt = 2 * kp_c + j
                            nc.tensor.matmul(
                                pso[:], vvt[:, kt, h * HD:(h + 1) * HD],
                                ex_c[:, j, :],
                                start=(kt == 0), stop=(kt == KT - 1))
                        t = accp.tile([P, SCHUNK], BF16, tag="pa1", bufs=4)
                        nc.vector.tensor_add(t[:], ex_c[:, 0, :], ex_c[:, 1, :])
                        l1.append(t)
                        if kp_c % 2 == 1:
                            t2 = accp.tile([P, SCHUNK], BF16, tag="pa2", bufs=4)
                            nc.vector.tensor_add(t2[:], l1[-2][:], l1[-1][:])
                            l2.append(t2)
                        if kp_c % 4 == 3:
                            t3 = accp.tile([P, SCHUNK], BF16, tag="pa3", bufs=4)
                            nc.vector.tensor_add(t3[:], l2[-2][:], l2[-1][:])
                            l3.append(t3)

                    prev = None
                    for kp in range(KT // 2):
                        pss = psp.tile([P, 2, SCHUNK], F32, tag="pss", bufs=2)
                        for j in range(2):
                            kt = 2 * kp + j
                            nc.tensor.matmul(
                                pss[:, j, :],
                                kTt[:, h, kt * P:(kt + 1) * P],
                                qTt[:, h, c * SCHUNK:(c + 1) * SCHUNK],
                                start=True, stop=True)
                        ex = expp.tile([P, 2, SCHUNK], BF16, tag="expP",
                                       bufs=6)
                        nc.scalar.activation(ex[:], pss[:], EXP,
                                             scale=INV_SQRT_HD)
                        if prev is not None:
                            consume(*prev)
                        prev = (kp, ex)

                    # wrap-up of the PREVIOUS head (its tree is a full head
                    # old -> no PE stall) + o_proj filler of the previous
                    # chunk bridge the last pair's ACT latency too
                    if wrapq:
                        wrapup()
                    pop_filler(4)
                    consume(*prev)
                    wrapq.append((ctx, h, pso, l3))

                for st in range(ST):
                    for ic in range(IC):
                        pending.append((1, lambda alt=False, ctx=ctx, c=c,
                                        st=st, ic=ic:
                                        oproj_group(ctx, c, st, ic, alt)))
            wrapup()  # chunk 3 head 3
            drain = 0
            while pending:
                _, fn = pending.pop(0)
                fn(alt=(drain % 3))
                drain += 1

    nc.finalize()
    return nc


def _bf16(a):
    return np.asarray(a, dtype=np.float32).astype(ml_dtypes.bfloat16)


def _pack_x(xT):
    """[D, S] bf16 -> [NG*P, DT*GR] blocked so device DMAs are contiguous."""
    return np.ascontiguousarray(
        xT.reshape(DT, P, NG, GR).transpose(2, 1, 0, 3)
    ).reshape(NG * P, DT * GR)


def _pack_w_mt(wT):
    """[D, DG] bf16 -> [MT*P, DT*HD] blocked by stationary column block."""
    return np.ascontiguousarray(
        wT.reshape(DT, P, MT, HD).transpose(2, 1, 0, 3)
    ).reshape(MT * P, DT * HD)


def kernel(hidden_states, wq, wk, wv, wo):
    global last_run
    if "nc" not in _cache:
        _cache["nc"] = build()
    nc = _cache["nc"]

    hidden_states = np.asarray(hidden_states, dtype=np.float32)
    wq = np.asarray(wq, dtype=np.float32)
    wk = np.asarray(wk, dtype=np.float32)
    wv = np.asarray(wv, dtype=np.float32)
    wo = np.asarray(wo, dtype=np.float32)

    xP = [_pack_x(_bf16(hidden_states[b].T)) for b in range(B)]
    in_maps = []
    for c in range(NCORES):
        b, g = divmod(c, G)
        sl = slice(g * DG, (g + 1) * DG)
        in_maps.append({
            "xP": xP[b],
            "wqT": _pack_w_mt(_bf16(wq[sl, :].T)),
            "wkT": _pack_w_mt(_bf16(wk[sl, :].T)),
            "wvT": _bf16(wv[sl, :].T),
            "woT": _bf16(wo[:, sl].T),
        })

    trace = os.environ.get("BASSKERNEL_TRACE", "0") == "1"
    last_run = run_bass_kernel_spmd(
        nc, in_maps, core_ids=list(range(NCORES)), trace=trace)

    out = np.empty((B, S, D), dtype=np.float32)
    for b in range(B):
        acc = None
        for g in range(G):
            part = last_run.results[b * G + g]["out"].astype(np.float32)
            acc = part if acc is None else acc + part
        out[b] = acc
    return out


# revision 44
# speedup vs baseline: 1.0198x; 1.0106x over previous
"""Multi-head attention (B=2, S=2048, D=2048, H=16) on 8 TRN2 NeuronCores.

Sharding: data-parallel over batch (2) x Megatron tensor-parallel over heads
(4 groups of 4 heads). Core c = 4*b + g handles batch b, heads [4g, 4g+4).
Host sums the 4 o_proj partials per batch and stacks the 2 batches.

Schedule (v3, all-bf16, single-pass x):
  Phase A: x streamed once in 4 granules of [P, 16dt, 512s]; per granule the
  k, v AND q projections all run (q no longer recomputed in phase B), outputs
  kept in SBUF as bf16: kT/qT [P, 4h, S], vv [P, 16st, DG]. All matmuls bf16
  (same PE rate as f32r, FWL weight loads, half the DMA/SBUF of the f32r
  baseline). Weights arrive on the scalar-engine DMA ring, x on the sync
  ring, so the first k-chain starts after ~400KB.

  Phase B per 512-query chunk, per head: scores matmuls write kt-PAIRS into a
  2-bank PSUM tile, ONE ACT exp covers the pair ([P,1024], amortizes the
  ~250ns ACT fixed cost), av matmul consumes each half (bf16). Softmax
  denominator: bf16 pair-sum tree on DVE (8+4+2 adds) then two ones-matmuls
  accumulate the partition reduction into a [128,512] PSUM tile -- every
  partition holds the sum, so reciprocal+normalize are full-width 128-lane
  DVE ops (the f32r baseline burned 54us of PE on per-kt ones-matmuls and
  3.3us/head on single-lane [1,512] reciprocals). o_proj bf16, out stored
  bf16 (host upcasts and sums partials in f32).

Further scheduling: av + denominator tree run one pair behind scores (PE
never waits on ACT's pair-exp latency); the denominator wrap-up of head h
is deferred into head h+1's slot; o_proj groups of chunk c-1 are emitted
between chunk c's heads as PE filler; the final drain rotates its PSUM
across three tags and copies on the idle scalar engine; ~4us of no-dep
warmup matmuls cross the HAM 3.41us activity window so real chains start
at 2.4GHz. fp8/DoubleRow was evaluated and rejected: e4m3's ~3.6% RMS
operand error puts every variant (5-10e-2) over the 2e-2 gate.

Per-core PE streaming floor is ~342us; measured HW exec ~384us
(vs 586us for the f32r baseline).
"""

import math
import os

import ml_dtypes
import numpy as np

import concourse.mybir as mybir
import concourse.tile as tile
from concourse import bacc
from concourse.bass_utils import run_bass_kernel_spmd

F32 = mybir.dt.float32
BF16 = mybir.dt.bfloat16
EXP = mybir.ActivationFunctionType.Exp

B, S, D = 2, 2048, 2048
H = 16
HD = 128
G = 4
HLOC = H // G          # 4 heads per core
DG = HLOC * HD         # 512
P = 128
NCORES = 8

DT = D // P            # 16 contraction tiles
GR = 512               # x granule (s columns)
NG = S // GR           # 4
SCHUNK = 512
QC = S // SCHUNK       # 4 query chunks
KT = S // P            # 16 key tiles
MT = DG // HD          # 4 stationary column blocks per projection
IC = D // SCHUNK       # 4
ST = GR // P           # 4
INV_SQRT_HD = 1.0 / math.sqrt(HD)

_cache = {}
last_run = None  # BassKernelResults of the most recent execution (for test.py)


def build():
    nc = bacc.Bacc(None, target_bir_lowering=False)

    xP_dr = nc.dram_tensor("xP", [NG * P, DT * GR], BF16, kind="ExternalInput")
    wqT_dr = nc.dram_tensor("wqT", [MT * P, DT * HD], BF16, kind="ExternalInput")
    wkT_dr = nc.dram_tensor("wkT", [MT * P, DT * HD], BF16, kind="ExternalInput")
    wvT_dr = nc.dram_tensor("wvT", [D, DG], BF16, kind="ExternalInput")
    woT_dr = nc.dram_tensor("woT", [DG, D], BF16, kind="ExternalInput")
    out_d = nc.dram_tensor("out", [S, D], BF16, kind="ExternalOutput")

    xP_v = xP_dr.rearrange("(g p) (o s) -> g p o s", p=P, s=GR)
    wqT_v = wqT_dr.rearrange("(m p) (o h) -> m p o h", p=P, h=HD)
    wkT_v = wkT_dr.rearrange("(m p) (o h) -> m p o h", p=P, h=HD)
    wvT_v = wvT_dr.rearrange("(o p) m -> p o m", p=P)
    woT_v = woT_dr.rearrange("(o p) i -> p o i", p=P)

    with tile.TileContext(nc) as tc:
        with (
            tc.tile_pool(name="persist", bufs=1) as persist,
            tc.tile_pool(name="wA", bufs=1) as wA,
            tc.tile_pool(name="xs", bufs=2) as xpool,
            tc.tile_pool(name="expp", bufs=4) as expp,
            tc.tile_pool(name="accp", bufs=4) as accp,
            tc.tile_pool(name="ctxp", bufs=2) as ctxp,
            tc.tile_pool(name="small", bufs=2) as small,
            tc.tile_pool(name="ps", bufs=1, space="PSUM") as psp,
        ):
            kTt = persist.tile([P, HLOC, S], BF16, tag="kT")
            qTt = persist.tile([P, HLOC, S], BF16, tag="qT")
            vvt = persist.tile([P, KT, DG], BF16, tag="vv")
            wot = persist.tile([P, MT, D], BF16, tag="wo")
            onesb = persist.tile([P, P], BF16, tag="ones")
            ones32 = persist.tile([P, P], F32, tag="ones32")
            nc.vector.memset(ones32[:], 1.0)
            nc.vector.tensor_copy(onesb[:], ones32[:])

            # ~4us of tiny matmuls bridging the initial DMA wait: keeps the
            # PE-HAM activity window busy so the real chains start at 2.4GHz
            # instead of paying ~3.4us of half-clock warmup mid-phase-A.
            # Lives in the pso tag, which is idle until phase B -- it must
            # NOT share rotation with the phase-A ps512 accumulators.
            # 40 x ~107ns (cold) = ~4.3us of continuous PE activity: enough
            # to cross the HAM 3.41us SHORT window so the un-throttle to
            # 2.4GHz fires before the first real chains start (28 was
            # measured to stay just under the window -> cold until ~30us)
            warm = psp.tile([P, SCHUNK], F32, tag="pso", bufs=2)
            for _ in range(40):
                nc.tensor.matmul(warm[0:64, 0:64], onesb[:, 0:64],
                                 onesb[:, 0:64], start=True, stop=True)

            wk_sb = wA.tile([P, MT, DT, HD], BF16, tag="wk")
            wq_sb = wA.tile([P, MT, DT, HD], BF16, tag="wq")
            wv_sb = wA.tile([P, DT, DG], BF16, tag="wv")

            # weights on the ACT DMA ring (idle until phase B), x on sync.
            # First k-chain's weights sliced fine so PE starts early.
            xg0 = xpool.tile([P, DT, GR], BF16, tag="xg")
            # wk mt0 in 2 coarse slices: the chain-major k loop only needs
            # dt0 early, and 8 fine slices cost 6.4us of issue time that
            # delays the x dt6+ slices queued behind them on this ring
            nc.scalar.dma_start(wk_sb[:, 0, 0:8], wkT_v[0, :, 0:8])
            nc.scalar.dma_start(wk_sb[:, 0, 8:16], wkT_v[0, :, 8:16])
            # granule 0's upper 10 dt-slices ride the scalar ring: the sync
            # ring's issue rate delivers ~6 slices by the time the ascending
            # k chain needs dt6, so dt6+ must come from the other ring
            for d0, w in [(6, 2), (8, 2), (10, 3), (13, 3)]:
                nc.scalar.dma_start(xg0[:, d0:d0 + w], xP_v[0, :, d0:d0 + w])
            for mt in range(1, MT):
                nc.scalar.dma_start(wk_sb[:, mt], wkT_v[mt])
            for d0 in range(0, DT, 4):
                nc.scalar.dma_start(wv_sb[:, d0:d0 + 4], wvT_v[:, d0:d0 + 4])
            for mt in range(MT):
                nc.scalar.dma_start(wq_sb[:, mt], wqT_v[mt])
            for jt in range(MT):
                nc.scalar.dma_start(wot[:, jt:jt + 1], woT_v[:, jt:jt + 1])

            # PE filler queue consumed between phase-B heads: weighted
            # closures (o_proj groups weight 1, deferred q-proj groups 4)
            pending = []

            def pop_filler(budget=4):
                while budget > 0 and pending:
                    w, fn = pending.pop(0)
                    fn()
                    budget -= w

            # ---------- phase A: k, v, q projections ----------
            for g in range(NG):
                if g == 0:
                    xg = xg0
                    for d0 in range(4):
                        nc.sync.dma_start(xg[:, d0:d0 + 1], xP_v[g, :, d0:d0 + 1])
                    nc.sync.dma_start(xg[:, 4:6], xP_v[g, :, 4:6])
                else:
                    xg = xpool.tile([P, DT, GR], BF16, tag="xg")
                    for d0 in range(0, DT, 4):
                        nc.sync.dma_start(xg[:, d0:d0 + 4], xP_v[g, :, d0:d0 + 4])

                for mt in range(MT):  # k projection
                    ps = psp.tile([P, GR], F32, tag="ps512", bufs=2)
                    for dt in range(DT):
                        nc.tensor.matmul(
                            ps[:], wk_sb[:, mt, dt, :], xg[:, dt, :],
                            start=(dt == 0), stop=(dt == DT - 1))
                        if g == 0 and mt == 0:
                            # no-dep filler: the first chain is DMA-paced,
                            # and a >3.4us PE idle here re-throttles HAM to
                            # 1.2GHz for the next 14us (observed) -- keep it
                            # busy (later chains run on resident x)
                            nc.tensor.matmul(warm[0:64, 0:64],
                                             onesb[:, 0:64], onesb[:, 0:64],
                                             start=True, stop=True)
                    nc.vector.tensor_copy(
                        kTt[:, mt, g * GR:(g + 1) * GR], ps[:])

                for st in range(ST):  # v projection
                    ps = psp.tile([P, DG], F32, tag="ps512", bufs=2)
                    for dt in range(DT):
                        nc.tensor.matmul(
                            ps[:], xg[:, dt, st * P:(st + 1) * P], wv_sb[:, dt, :],
                            start=(dt == 0), stop=(dt == DT - 1))
                    nc.vector.tensor_copy(vvt[:, g * ST + st, :], ps[:])

                def q_group(mt, xg=xg, g=g, tag="ps512"):
                    ps = psp.tile([P, GR], F32, tag=tag, bufs=2)
                    for dt in range(DT):
                        nc.tensor.matmul(
                            ps[:], wq_sb[:, mt, dt, :], xg[:, dt, :],
                            start=(dt == 0), stop=(dt == DT - 1))
                    nc.vector.tensor_copy(
                        qTt[:, mt, g * GR:(g + 1) * GR], ps[:])

                for mt in range(MT):  # q projection
                    q_group(mt)

            # ---------- phase B: attention + o_proj ----------
            def oproj_group(ctx_c, c, st, ic, alt=0):
                stile = c * ST + st
                if alt == 1:
                    # final drain: attention PSUM banks are idle, rotate
                    # through pss/pso too so consecutive groups pipeline
                    ps2 = psp.tile([P, 2, SCHUNK], F32, tag="pss", bufs=2)
                    ps = ps2[:, 0, :]
                elif alt == 2:
                    ps = psp.tile([P, SCHUNK], F32, tag="pso", bufs=2)
                else:
                    ps = psp.tile([P, SCHUNK], F32, tag="ps512", bufs=2)
                for jt in range(MT):
                    nc.tensor.matmul(
                        ps[:], ctx_c[:, jt, st * P:(st + 1) * P],
                        wot[:, jt, ic * SCHUNK:(ic + 1) * SCHUNK],
                        start=(jt == 0), stop=(jt == MT - 1))
                ob = small.tile([P, SCHUNK], BF16, tag="ostage", bufs=3)
                if alt:
                    # drain: DVE is the only busy engine left; ACT is idle
                    nc.scalar.copy(ob[:], ps[:])
                else:
                    nc.vector.tensor_copy(ob[:], ps[:])
                # sync HWDGE ring is idle in phase B (x loads done) and
                # avoids the multi-us SWDGE drain at teardown
                nc.sync.dma_start(
                    out_d[stile * P:(stile + 1) * P,
                          ic * SCHUNK:(ic + 1) * SCHUNK],
                    ob[:])

            # o_proj of chunk c-1 is issued interleaved between the heads of
            # chunk c: its ps512 tiles then rotate between the psden tiles
            # instead of queueing behind all four of them, and its matmuls
            # give PE filler work at every head boundary
            pending = []  # (ctx, c, st, ic) o_proj groups not yet issued
            wrapq = []    # (ctx, h, pso, l3) denominator wrap-ups not issued

            def wrapup():
                """ones-matmuls + reciprocal + normalize for the OLDEST
                pending head. Deferred by one head so the DVE tree is long
                done when the PE reaches the ones-matmuls (no PE stall)."""
                ctx_w, h_w, pso_w, l3_w = wrapq.pop(0)
                psden = psp.tile([P, SCHUNK], F32, tag="ps512", bufs=2)
                for i, t in enumerate(l3_w):
                    nc.tensor.matmul(psden[:], onesb[:], t[:],
                                     start=(i == 0), stop=(i == len(l3_w) - 1))
                rb = small.tile([P, SCHUNK], F32, tag="rb")
                # ~51 ULP is plenty for a softmax denominator; the exact
                # DVE reciprocal costs 3.4us/call and stalled the pso
                # PSUM pipeline
                nc.vector.reciprocal_approx_fast(rb[:], psden[:])
                nc.vector.tensor_mul(ctx_w[:, h_w, :], pso_w[:], rb[:])

            for c in range(QC):
                ctx = ctxp.tile([P, HLOC, SCHUNK], BF16, tag="ctx")
                for h in range(HLOC):
                    pso = psp.tile([P, SCHUNK], F32, tag="pso", bufs=2)
                    # av + denominator tree run one PAIR behind the scores
                    # so the PE never waits on ACT's ~1.1us pair-exp latency
                    l1, l2, l3 = [], [], []

                    def consume(kp_c, ex_c):
                        for j in range(2):
                            kt = 2 * kp_c + j
                            nc.tensor.matmul(
                                pso[:], vvt[:, kt, h * HD:(h + 1) * HD],
                                ex_c[:, j, :],
                                start=(kt == 0), stop=(kt == KT - 1))
                        t = accp.tile([P, SCHUNK], BF16, tag="pa1", bufs=4)
                        nc.vector.tensor_add(t[:], ex_c[:, 0, :], ex_c[:, 1, :])
                        l1.append(t)
                        if kp_c % 2 == 1:
                            t2 = accp.tile([P, SCHUNK], BF16, tag="pa2", bufs=4)
                            nc.vector.tensor_add(t2[:], l1[-2][:], l1[-1][:])
                            l2.append(t2)
                        if kp_c % 4 == 3:
                            t3 = accp.tile([P, SCHUNK], BF16, tag="pa3", bufs=4)
                            nc.vector.tensor_add(t3[:], l2[-2][:], l2[-1][:])
                            l3.append(t3)

                    prev = None
                    for kp in range(KT // 2):
                        pss = psp.tile([P, 2, SCHUNK], F32, tag="pss", bufs=2)
                        for j in range(2):
                            kt = 2 * kp + j
                            nc.tensor.matmul(
                                pss[:, j, :],
                                kTt[:, h, kt * P:(kt + 1) * P],
                                qTt[:, h, c * SCHUNK:(c + 1) * SCHUNK],
                                start=True, stop=True)
                        ex = expp.tile([P, 2, SCHUNK], BF16, tag="expP",
                                       bufs=6)
                        nc.scalar.activation(ex[:], pss[:], EXP,
                                             scale=INV_SQRT_HD)
                        if prev is not None:
                            consume(*prev)
                        prev = (kp, ex)

                    # wrap-up of the PREVIOUS head (its tree is a full head
                    # old -> no PE stall) + o_proj filler of the previous
                    # chunk bridge the last pair's ACT latency too
                    if wrapq:
                        wrapup()
                    pop_filler(4)
                    consume(*prev)
                    wrapq.append((ctx, h, pso, l3))

                for st in range(ST):
                    for ic in range(IC):
                        pending.append((1, lambda alt=False, ctx=ctx, c=c,
                                        st=st, ic=ic:
                                        oproj_group(ctx, c, st, ic, alt)))
            wrapup()  # chunk 3 head 3
            drain = 0
            while pending:
                _, fn = pending.pop(0)
                fn(alt=(drain % 3))
                drain += 1

    nc.finalize()
    return nc


def _bf16(a):
    return np.asarray(a, dtype=np.float32).astype(ml_dtypes.bfloat16)


def _pack_x(xT):
    """[D, S] bf16 -> [NG*P, DT*GR] blocked so device DMAs are contiguous."""
    return np.ascontiguousarray(
        xT.reshape(DT, P, NG, GR).transpose(2, 1, 0, 3)
    ).reshape(NG * P, DT * GR)


def _pack_w_mt(wT):
    """[D, DG] bf16 -> [MT*P, DT*HD] blocked by stationary column block."""
    return np.ascontiguousarray(
        wT.reshape(DT, P, MT, HD).transpose(2, 1, 0, 3)
    ).reshape(MT * P, DT * HD)


def kernel(hidden_states, wq, wk, wv, wo):
    global last_run
    if "nc" not in _cache:
        _cache["nc"] = build()
    nc = _cache["nc"]

    hidden_states = np.asarray(hidden_states, dtype=np.float32)
    wq = np.asarray(wq, dtype=np.float32)
    wk = np.asarray(wk, dtype=np.float32)
    wv = np.asarray(wv, dtype=np.float32)
    wo = np.asarray(wo, dtype=np.float32)

    xP = [_pack_x(_bf16(hidden_states[b].T)) for b in range(B)]
    in_maps = []
    for c in range(NCORES):
        b, g = divmod(c, G)
        sl = slice(g * DG, (g + 1) * DG)
        in_maps.append({
            "xP": xP[b],
            "wqT": _pack_w_mt(_bf16(wq[sl, :].T)),
            "wkT": _pack_w_mt(_bf16(wk[sl, :].T)),
            "wvT": _bf16(wv[sl, :].T),
            "woT": _bf16(wo[:, sl].T),
        })

    trace = os.environ.get("BASSKERNEL_TRACE", "0") == "1"
    last_run = run_bass_kernel_spmd(
        nc, in_maps, core_ids=list(range(NCORES)), trace=trace)

    out = np.empty((B, S, D), dtype=np.float32)
    for b in range(B):
        acc = None
        for g in range(G):
            part = last_run.results[b * G + g]["out"].astype(np.float32)
            acc = part if acc is None else acc + part
        out[b] = acc
    return out
